# revision 1
# baseline (speedup 1.0000x reference)
"""Trainium2 Bass kernel for the CRAFT-style hard-negative-mining MSE loss.

Reference math (per branch, over N = 16*768*768 flat pixels):
    all_loss = (pred - target)^2
    pos_mask = (target >= 0.3) & (weight != 0)
    neg_mask = (target < 0.1)
    pos_sum  = sum(pos_mask * all_loss * weight)
    k        = min(max(1000, 3*num_pos), num_neg)
    topk_sum = sum of k largest all_loss among negatives
    loss     = (pos_sum + topk_sum) / (num_pos + k)
    out      = loss_char + loss_aff

With uniform targets num_pos ~ 0.7*N, so 3*num_pos >> num_neg and
k == num_neg: the top-k degenerates to the full sum over negatives.
The device kernel computes, per 1/8 shard, per branch:
    S1 = sum(neg_mask * all_loss)          (DVE scalar_tensor_tensor+accum)
    S2 = sum((t>=0.3) * all_loss * weight) (DVE, (w!=0) is absorbed by *w)
    sumsign_neg = sum(sign(0.1 - t))       (ScalarE Sign activation+accum)
    sumsign_pos = sum(sign(t - 0.3))       (ScalarE Sign activation+accum)
Counts follow exactly from the sign sums (thresholds 0.1f/0.3f are not
bf16-representable, so sign is never 0). The host merges the 8 shards and
applies the k/denominator logic; a full numpy fallback covers the
(never-hit-here) k < num_neg case.

Inputs are cast to bf16 on the host: halves HBM traffic and doubles DVE
throughput; measured end-to-end relative error is ~1e-4 (dominated by
threshold reclassification of ~0.05% of pixels near 0.1/0.3).

All six shard tensors are packed into one [P, 6, FD] DRAM tensor per core
so each tile iteration needs a single DMA (instructions on trn2 have very
few semaphore-wait slots; one DMA semaphore per iteration keeps every
consumer at <=1 wait; Bacc.compile()'s generate_event_semaphores splits
the rest).
"""

import os
import numpy as np
import ml_dtypes

N_CORES = 8
B, H, W = 16, 768, 768
NPX = B * H * W              # 9_437_184 flat pixels
P = 128                      # SBUF partitions
FD = NPX // (N_CORES * P)    # 9216 free-dim elements per core per tensor
N_TILES = 4                  # tiles per branch
F = FD // N_TILES            # tile width

USE_BF16 = os.environ.get("KERNEL_FP32", "0") != "1"

THRESH_NEG = 0.1
THRESH_POS = 0.3

# quantity indices in the accumulators
Q_S1, Q_S2 = 0, 1            # DVE accumulator columns
Q_SGN_NEG, Q_SGN_POS = 0, 1  # ACT accumulator columns

_compiled = None             # cached nc
LAST_RESULTS = None          # BassKernelResults of the last run (for profiling)


def _build_nc():
    import concourse.bacc as bacc
    import concourse.mybir as mybir
    import concourse.tile as tile
    from contextlib import ExitStack

    DT = mybir.dt.bfloat16 if USE_BF16 else mybir.dt.float32
    f32 = mybir.dt.float32
    Alu = mybir.AluOpType
    Act = mybir.ActivationFunctionType

    nc = bacc.Bacc(
        "TRN2",
        target_bir_lowering=False,
        debug=False,
        num_devices=N_CORES,
    )

    # bias constants for the Sign activations, registered pre-Tile like
    # Bass's own const APs (memset + barrier; no Tile-tracked deps)
    bias_neg_t = nc.alloc_sbuf_tensor("bias_neg_c", [P, 1], f32)
    nc.gpsimd.memset(bias_neg_t.ap(), THRESH_NEG)
    bias_pos_t = nc.alloc_sbuf_tensor("bias_pos_c", [P, 1], f32)
    nc.gpsimd.memset(bias_pos_t.ap(), -THRESH_POS)
    nc.all_engine_barrier()
    bias_neg = bias_neg_t.ap()
    bias_pos = bias_pos_t.ap()

    # packed input: dim1 = (p_c, t_c, w_c, p_a, t_a, w_a)
    pk = nc.declare_dram_parameter("pk", [P, 6, FD], DT, isOutput=False)
    out_dve = nc.declare_dram_parameter("acc_dve", [P, 2 * 2 * N_TILES], f32, isOutput=True)
    out_act = nc.declare_dram_parameter("acc_act", [P, 2 * 2 * N_TILES], f32, isOutput=True)

    with tile.TileContext(nc) as tc, ExitStack() as ctx:
        in_pool = ctx.enter_context(tc.tile_pool(name="in", bufs=3))
        tmp_pool = ctx.enter_context(tc.tile_pool(name="tmp", bufs=2))
        acc_pool = ctx.enter_context(tc.tile_pool(name="acc", bufs=1))

        acc_dve = acc_pool.tile([P, 2 * 2 * N_TILES], f32, tag="acc_dve")
        acc_act = acc_pool.tile([P, 2 * 2 * N_TILES], f32, tag="acc_act")

        for b in range(2):
            for i in range(N_TILES):
                sl = slice(i * F, (i + 1) * F)
                tin = in_pool.tile([P, 3, F], DT, tag="in")
                nc.sync.dma_start(tin[:], pk[:, 3 * b : 3 * b + 3, sl])
                pt = tin[:, 0, :]
                tt = tin[:, 1, :]
                wt = tin[:, 2, :]

                def dcol(q):
                    j = (b * 2 + q) * N_TILES + i
                    return acc_dve[:, j : j + 1]

                def acol(q):
                    j = (b * 2 + q) * N_TILES + i
                    return acc_act[:, j : j + 1]

                # d = pred - target            (DVE)
                d = tmp_pool.tile([P, F], DT, tag="d")
                nc.vector.tensor_tensor(d[:], pt, tt, Alu.subtract)
                # l = d^2                      (ScalarE)
                l = tmp_pool.tile([P, F], DT, tag="l")
                nc.scalar.activation(l[:], d[:], Act.Square)
                # lw = l * w                   (DVE)
                lw = tmp_pool.tile([P, F], DT, tag="lw")
                nc.vector.tensor_tensor(lw[:], l[:], wt, Alu.mult)
                # S1 += sum((t < 0.1) * l)     (DVE fused mask+mul+reduce)
                scr_d = tmp_pool.tile([P, F], DT, tag="scr_d")
                nc.vector.scalar_tensor_tensor(
                    scr_d[:], tt, THRESH_NEG, l[:], Alu.is_lt, Alu.mult,
                    accum_out=dcol(Q_S1),
                )
                # S2 += sum((t >= 0.3) * l * w)
                nc.vector.scalar_tensor_tensor(
                    scr_d[:], tt, THRESH_POS, lw[:], Alu.is_ge, Alu.mult,
                    accum_out=dcol(Q_S2),
                )
                # sumsign_neg += sum(sign(0.1 - t))   (ScalarE)
                scr_a = tmp_pool.tile([P, F], DT, tag="scr_a")
                nc.scalar.activation(
                    scr_a[:], tt, Act.Sign, bias=bias_neg, scale=-1.0,
                    accum_out=acol(Q_SGN_NEG),
                )
                # sumsign_pos += sum(sign(t - 0.3))   (ScalarE)
                nc.scalar.activation(
                    scr_a[:], tt, Act.Sign, bias=bias_pos, scale=1.0,
                    accum_out=acol(Q_SGN_POS),
                )

        nc.sync.dma_start(out_dve[:], acc_dve[:])
        nc.sync.dma_start(out_act[:], acc_act[:])

    nc.compile()
    return nc


def _get_nc():
    global _compiled
    if _compiled is None:
        _compiled = _build_nc()
    return _compiled


def _np_branch_fallback(pred, target, weight):
    """Exact reference math in numpy float64 (handles k < num_neg)."""
    pred = pred.astype(np.float64)
    target = target.astype(np.float64)
    weight = weight.astype(np.float64)
    all_loss = (pred - target) ** 2
    pos_mask = (target >= THRESH_POS) & (weight != 0)
    neg_mask = target < THRESH_NEG
    pos_sum = float(np.sum(np.where(pos_mask, all_loss * weight, 0.0)))
    num_pos = int(np.sum(pos_mask))
    num_neg = int(np.sum(neg_mask))
    k = min(max(1000, 3 * num_pos), num_neg)
    neg_vals = all_loss[neg_mask]
    if k >= num_neg:
        topk = float(neg_vals.sum())
    elif k <= 0:
        topk = 0.0
    else:
        topk = float(np.partition(neg_vals, num_neg - k)[num_neg - k :].sum())
    return (pos_sum + topk) / (num_pos + k)


def kernel(output, character_map, affinity_map, character_weight, affinity_weight):
    from concourse.bass_utils import run_bass_kernel_spmd

    global LAST_RESULTS
    np_dt = ml_dtypes.bfloat16 if USE_BF16 else np.float32

    output = np.asarray(output, dtype=np.float32)

    def shard(a):
        # flat pixel order (b, h, w) -> [core, partition, free]
        return np.ascontiguousarray(a).reshape(N_CORES, P, FD).astype(np_dt)

    packed = np.empty((N_CORES, P, 6, FD), dtype=np_dt)
    packed[:, :, 0] = shard(output[:, 0])
    packed[:, :, 1] = shard(np.asarray(character_map, dtype=np.float32))
    packed[:, :, 2] = shard(np.asarray(character_weight, dtype=np.float32))
    packed[:, :, 3] = shard(output[:, 1])
    packed[:, :, 4] = shard(np.asarray(affinity_map, dtype=np.float32))
    packed[:, :, 5] = shard(np.asarray(affinity_weight, dtype=np.float32))

    in_maps = [{"pk": packed[c]} for c in range(N_CORES)]

    nc = _get_nc()
    res = run_bass_kernel_spmd(
        nc,
        in_maps,
        list(range(N_CORES)),
        trace=os.environ.get("KERNEL_TRACE", "0") == "1",
    )
    LAST_RESULTS = res

    # [cores, P, branch, quantity, tile] -> sum over cores, partitions, tiles
    acc_dve = np.stack([r["acc_dve"] for r in res.results]).astype(np.float64)
    acc_act = np.stack([r["acc_act"] for r in res.results]).astype(np.float64)
    sums_dve = acc_dve.reshape(N_CORES, P, 2, 2, N_TILES).sum(axis=(0, 1, 4))
    sums_act = acc_act.reshape(N_CORES, P, 2, 2, N_TILES).sum(axis=(0, 1, 4))

    total = 0.0
    for bidx, (tmap, wmap) in enumerate(
        [(character_map, character_weight), (affinity_map, affinity_weight)]
    ):
        s1 = sums_dve[bidx, Q_S1]
        s2 = sums_dve[bidx, Q_S2]
        num_neg = int(round((sums_act[bidx, Q_SGN_NEG] + NPX) / 2))
        num_pos = int(round((sums_act[bidx, Q_SGN_POS] + NPX) / 2))
        k = min(max(1000, 3 * num_pos), num_neg)
        if k == num_neg:
            total += (s2 + s1) / (num_pos + k)
        else:
            # top-k actually selective: fall back to exact host computation
            total += _np_branch_fallback(
                output[:, bidx].reshape(-1),
                np.asarray(tmap, dtype=np.float32).reshape(-1),
                np.asarray(wmap, dtype=np.float32).reshape(-1),
            )

    return np.float32(total)



# revision 5
# speedup vs baseline: 1.0328x; 1.0328x over previous
"""Trainium2 Bass kernel for the CRAFT-style hard-negative-mining MSE loss.

Reference math (per branch, over N = 16*768*768 flat pixels):
    all_loss = (pred - target)^2
    pos_mask = (target >= 0.3) & (weight != 0)
    neg_mask = (target < 0.1)
    pos_sum  = sum(pos_mask * all_loss * weight)
    k        = min(max(1000, 3*num_pos), num_neg)
    topk_sum = sum of k largest all_loss among negatives
    loss     = (pos_sum + topk_sum) / (num_pos + k)
    out      = loss_char + loss_aff

With uniform targets num_pos ~ 0.7*N, so 3*num_pos >> num_neg and
k == num_neg: the top-k degenerates to the full sum over negatives.

Device strategy (v2, TensorE-assisted): per 1/8 shard, per branch-tile
[128, F]:
    DVE:  m_neg = (t < 0.1)            tensor_scalar is_lt  (4x mode)
          m_pos = (t >= 0.3)           tensor_scalar is_ge  (4x mode)
            each with accum_out giving the counts num_neg/num_pos
          d  = p - t                    tensor_tensor        (2x mode)
          mw = m_pos * w                tensor_tensor        (2x mode)
    ACT:  l  = d^2                      Square               (1x, only op)
    PE:   for each 128-col block k: psum += l_blk^T @ [m_neg_blk | mw_blk]
          accumulated over the whole branch into one [128, 256] PSUM
          region.  diag(psum[:, 0:128])   = per-col <m_neg, l> -> S1
          diag(psum[:, 128:256]) = per-col <mw, l>    -> S2
The [128, 256] PSUM regions are copied to SBUF (ScalarE) and DMA'd out;
the host takes the two diagonals, sums them with the counts across the
8 shards, and applies the k/denominator logic (with a full numpy
fallback for the never-hit-here k < num_neg case).

This moves the two big masked-sum passes off the DVE (where
scalar_tensor_tensor only runs at 1x) onto the otherwise-idle TensorE,
and the two count passes off ScalarE Sign activations (1x) onto DVE
tensor_scalar compares (4x).  Inputs are cast to bf16 on the host:
halves HBM traffic and doubles DVE tensor_tensor throughput.

All three shard tensors per branch-tile are packed contiguously so each
tile iteration needs a single DMA with 13.8KB-contiguous
per-partition chunks.
"""

import os
import numpy as np
import ml_dtypes

N_CORES = 8
B, H, W = 16, 768, 768
NPX = B * H * W              # 9_437_184 flat pixels
P = 128                      # SBUF partitions
FD = NPX // (N_CORES * P)    # 9216 free-dim elements per core per tensor
N_TILES = 4                  # tiles per branch
F = FD // N_TILES            # 2304 tile width
NBLK = F // P                # 18 matmul blocks per tile

THRESH_NEG = 0.1
THRESH_POS = 0.3

Q_NEG, Q_POS = 0, 1          # count accumulator columns

_compiled = None             # cached nc
LAST_RESULTS = None          # BassKernelResults of the last run (for profiling)


def _build_nc():
    import concourse.bacc as bacc
    import concourse.mybir as mybir
    import concourse.tile as tile
    from contextlib import ExitStack

    DT = mybir.dt.bfloat16
    f32 = mybir.dt.float32
    Alu = mybir.AluOpType
    Act = mybir.ActivationFunctionType

    nc = bacc.Bacc(
        "TRN2",
        target_bir_lowering=False,
        debug=False,
        num_devices=N_CORES,
    )

    # packed input: [P, branch, tile, (p,t,w), F]
    pk = nc.declare_dram_parameter("pk", [P, 2, N_TILES, 3, F], DT, isOutput=False)
    out_cnt = nc.declare_dram_parameter("acc_cnt", [P, 2 * 2 * N_TILES], f32, isOutput=True)
    out_ps = nc.declare_dram_parameter("acc_ps", [P, 2, 2 * P], f32, isOutput=True)

    with tile.TileContext(nc) as tc, ExitStack() as ctx:
        in_pool = ctx.enter_context(tc.tile_pool(name="in", bufs=3))
        d_pool = ctx.enter_context(tc.tile_pool(name="d", bufs=2))
        l_pool = ctx.enter_context(tc.tile_pool(name="l", bufs=2))
        m_pool = ctx.enter_context(tc.tile_pool(name="m", bufs=2))
        acc_pool = ctx.enter_context(tc.tile_pool(name="acc", bufs=1))
        ps_pool = ctx.enter_context(tc.psum_pool(name="ps", bufs=1))

        acc_cnt = acc_pool.tile([P, 2 * 2 * N_TILES], f32, tag="acc_cnt")
        ps_sb = acc_pool.tile([P, 2, 2 * P], f32, tag="ps_sb")
        psum = [
            ps_pool.tile([P, 2 * P], f32, tag=f"psum{b}", name=f"psum{b}")
            for b in range(2)
        ]

        for b in range(2):
            for i in range(N_TILES):
                tin = in_pool.tile([P, 3, F], DT, tag="in")
                nc.sync.dma_start(tin[:], pk[:, b, i])
                pt = tin[:, 0, :]
                tt = tin[:, 1, :]
                wt = tin[:, 2, :]

                def ccol(q, b=b, i=i):
                    j = (b * 2 + q) * N_TILES + i
                    return acc_cnt[:, j : j + 1]

                # m[:,0,:] = (t < 0.1), count -> num_neg     (DVE TS 4x)
                m = m_pool.tile([P, 2, F], DT, tag="m")
                nc.vector.tensor_scalar(
                    m[:, 0, :], tt, THRESH_NEG, 0.0, Alu.is_lt, Alu.add,
                    accum_out=ccol(Q_NEG),
                )
                # scr = (t >= 0.3), count -> num_pos         (DVE TS 4x)
                scr = d_pool.tile([P, F], DT, tag="scr")
                nc.vector.tensor_scalar(
                    scr, tt, THRESH_POS, 0.0, Alu.is_ge, Alu.add,
                    accum_out=ccol(Q_POS),
                )
                # d = pred - target                          (DVE TT 2x)
                d = d_pool.tile([P, F], DT, tag="d")
                nc.vector.tensor_tensor(d[:], pt, tt, Alu.subtract)
                # l = d^2                                    (ACT Square 1x)
                l = l_pool.tile([P, F], DT, tag="l")
                nc.scalar.activation(l[:], d[:], Act.Square)
                # m[:,1,:] = m_pos * w                       (DVE TT 2x)
                nc.vector.tensor_tensor(m[:, 1, :], scr[:], wt, Alu.mult)

                # psum[b] += l_blk^T @ [m_neg_blk | mw_blk]  (PE)
                for k in range(NBLK):
                    sl = slice(k * P, (k + 1) * P)
                    nc.tensor.matmul(
                        psum[b][:, :],
                        l[:, sl],
                        m[:, :, sl],
                        start=(i == 0 and k == 0),
                        stop=(i == N_TILES - 1 and k == NBLK - 1),
                    )

            # dump the accumulated [128, 256] PSUM region to SBUF (ScalarE)
            nc.scalar.copy(ps_sb[:, b, :], psum[b][:, :])

        nc.sync.dma_start(out_cnt[:], acc_cnt[:])
        nc.sync.dma_start(out_ps[:], ps_sb[:])

    nc.compile()
    return nc


def _get_nc():
    global _compiled
    if _compiled is None:
        _compiled = _build_nc()
    return _compiled


def _np_branch_fallback(pred, target, weight):
    """Exact reference math in numpy float64 (handles k < num_neg)."""
    pred = pred.astype(np.float64)
    target = target.astype(np.float64)
    weight = weight.astype(np.float64)
    all_loss = (pred - target) ** 2
    pos_mask = (target >= THRESH_POS) & (weight != 0)
    neg_mask = target < THRESH_NEG
    pos_sum = float(np.sum(np.where(pos_mask, all_loss * weight, 0.0)))
    num_pos = int(np.sum(pos_mask))
    num_neg = int(np.sum(neg_mask))
    k = min(max(1000, 3 * num_pos), num_neg)
    neg_vals = all_loss[neg_mask]
    if k >= num_neg:
        topk = float(neg_vals.sum())
    elif k <= 0:
        topk = 0.0
    else:
        topk = float(np.partition(neg_vals, num_neg - k)[num_neg - k :].sum())
    return (pos_sum + topk) / (num_pos + k)


def kernel(output, character_map, affinity_map, character_weight, affinity_weight):
    from concourse.bass_utils import run_bass_kernel_spmd

    global LAST_RESULTS
    np_dt = ml_dtypes.bfloat16

    output = np.asarray(output, dtype=np.float32)

    def shard(a):
        # flat pixel order (b, h, w) -> [core, partition, tile, free]
        return (
            np.ascontiguousarray(a)
            .reshape(N_CORES, P, N_TILES, F)
            .astype(np_dt)
        )

    packed = np.empty((N_CORES, P, 2, N_TILES, 3, F), dtype=np_dt)
    packed[:, :, 0, :, 0] = shard(output[:, 0])
    packed[:, :, 0, :, 1] = shard(np.asarray(character_map, dtype=np.float32))
    packed[:, :, 0, :, 2] = shard(np.asarray(character_weight, dtype=np.float32))
    packed[:, :, 1, :, 0] = shard(output[:, 1])
    packed[:, :, 1, :, 1] = shard(np.asarray(affinity_map, dtype=np.float32))
    packed[:, :, 1, :, 2] = shard(np.asarray(affinity_weight, dtype=np.float32))

    in_maps = [{"pk": packed[c]} for c in range(N_CORES)]

    nc = _get_nc()
    res = run_bass_kernel_spmd(
        nc,
        in_maps,
        list(range(N_CORES)),
        trace=os.environ.get("KERNEL_TRACE", "0") == "1",
    )
    LAST_RESULTS = res

    acc_cnt = np.stack([r["acc_cnt"] for r in res.results]).astype(np.float64)
    acc_ps = np.stack([r["acc_ps"] for r in res.results]).astype(np.float64)
    # counts: [cores, P, branch, quantity, tile] -> [branch, quantity]
    cnts = acc_cnt.reshape(N_CORES, P, 2, 2, N_TILES).sum(axis=(0, 1, 4))
    # masked sums: diag of psum[c] (S1 at cols 0:128, S2 at 128:256)
    idx = np.arange(P)
    s1 = acc_ps[:, idx, :, idx].sum(axis=(0, 1))          # [branch]
    s2 = acc_ps[:, idx, :, P + idx].sum(axis=(0, 1))      # [branch]

    total = 0.0
    for bidx, (tmap, wmap) in enumerate(
        [(character_map, character_weight), (affinity_map, affinity_weight)]
    ):
        num_neg = int(round(cnts[bidx, Q_NEG]))
        num_pos = int(round(cnts[bidx, Q_POS]))
        k = min(max(1000, 3 * num_pos), num_neg)
        if k == num_neg:
            total += (s1[bidx] + s2[bidx]) / (num_pos + k)
        else:
            # top-k actually selective: fall back to exact host computation
            total += _np_branch_fallback(
                output[:, bidx].reshape(-1),
                np.asarray(tmap, dtype=np.float32).reshape(-1),
                np.asarray(wmap, dtype=np.float32).reshape(-1),
            )

    return np.float32(total)


# revision 11
# speedup vs baseline: 1.2598x; 1.2198x over previous
"""Trainium2 Bass kernel for the CRAFT-style hard-negative-mining MSE loss.

Reference math (per branch, over N = 16*768*768 flat pixels):
    all_loss = (pred - target)^2
    pos_mask = (target >= 0.3) & (weight != 0)
    neg_mask = (target < 0.1)
    pos_sum  = sum(pos_mask * all_loss * weight)
    k        = min(max(1000, 3*num_pos), num_neg)
    topk_sum = sum of k largest all_loss among negatives
    loss     = (pos_sum + topk_sum) / (num_pos + k)
    out      = loss_char + loss_aff

With uniform targets num_pos ~ 0.7*N, so 3*num_pos >> num_neg and
k == num_neg: the top-k degenerates to the full sum over negatives.

Device strategy (v3, TensorE-assisted): per 1/8 shard, per branch-tile
[128, F=2304]:
    DVE:  m_neg = (t < 0.1)            tensor_scalar is_lt  (4x mode)
          m_pos = (t >= 0.3)           tensor_scalar is_ge  (4x mode)
          d  = p - t                    tensor_tensor        (2x mode)
          mw = m_pos * w                tensor_tensor        (2x mode)
    ACT:  l  = d^2                      Square (1x) -> strided [24, 96]
    GPS:  ones column at lext[:, :, 96] (tiny memset)
    PE:   24 blocks of 96 data cols:
            psum[0:97, 0:288] += [l_96 | ones]^T @ [m_neg | m_pos | mw]
          accumulated over the whole branch (96 matmuls) into one
          [97, 288] PSUM region:
            diag(rows 0:96 of group 0) -> per-col <m_neg, l> = S1
            diag(rows 0:96 of group 2) -> per-col <mw,    l> = S2
            row 96 of group 0          -> per-col sums of m_neg = num_neg
            row 96 of group 1          -> per-col sums of m_pos = num_pos
The [97, 288] PSUM regions are copied to SBUF (ScalarE) and DMA'd out;
the host extracts diagonals/count-rows, sums across the 8 shards, and
applies the k/denominator logic (with a full numpy fallback for the
never-hit-here k < num_neg case).

This moves the masked-sum and count reductions off DVE/ACT (where they
only run at 1x) onto the otherwise-idle TensorE.  Inputs are cast to
bf16 on the host: halves HBM traffic and doubles DVE tensor_tensor
throughput.
"""

import os
import numpy as np
import ml_dtypes

N_CORES = 8
B, H, W = 16, 768, 768
NPX = B * H * W              # 9_437_184 flat pixels
P = 128                      # SBUF partitions
FD = NPX // (N_CORES * P)    # 9216 free-dim elements per core per tensor
N_TILES = 4                  # tiles per branch
F = FD // N_TILES            # 2304 tile width
BD = 96                      # data columns per matmul block
NBLK = F // BD               # 24 matmul blocks per tile
PR = BD + 1                  # psum rows used (96 data + 1 count row)

THRESH_NEG = 0.1
THRESH_POS = 0.3

_compiled = None             # cached nc
LAST_RESULTS = None          # BassKernelResults of the last run (for profiling)


def _build_nc():
    import concourse.bacc as bacc
    import concourse.mybir as mybir
    import concourse.tile as tile
    from contextlib import ExitStack

    DT = mybir.dt.bfloat16
    f32 = mybir.dt.float32
    Alu = mybir.AluOpType
    Act = mybir.ActivationFunctionType

    nc = bacc.Bacc(
        "TRN2",
        target_bir_lowering=False,
        debug=False,
        num_devices=N_CORES,
    )

    # packed input: [P, branch, tile, (p,t,w), F]
    pk = nc.declare_dram_parameter("pk", [P, 2, N_TILES, 3, F], DT, isOutput=False)
    out_ps = nc.declare_dram_parameter("acc_ps", [PR, 2, 3, BD], f32, isOutput=True)

    with tile.TileContext(nc) as tc, ExitStack() as ctx:
        in_pool = ctx.enter_context(tc.tile_pool(name="in", bufs=3))
        d_pool = ctx.enter_context(tc.tile_pool(name="d", bufs=2))
        m_pool = ctx.enter_context(tc.tile_pool(name="m", bufs=2))
        acc_pool = ctx.enter_context(tc.tile_pool(name="acc", bufs=1))
        ps_pool = ctx.enter_context(tc.psum_pool(name="ps", bufs=1))

        ps_sb = acc_pool.tile([PR, 2, 3, BD], f32, tag="ps_sb")
        psum = [
            ps_pool.tile([PR, 3, BD], f32, tag=f"psum{b}", name=f"psum{b}")
            for b in range(2)
        ]
        # persistent double-buffered [l | ones] stationary tensors; the
        # ones column (col 96 of each 97-block) is written once up front
        # and never touched again
        lexts = [
            acc_pool.tile([P, NBLK, PR], DT, tag=f"lext{j}", name=f"lext{j}")
            for j in range(2)
        ]
        for j in range(2):
            nc.gpsimd.memset(lexts[j][:, :, BD : BD + 1], 1.0)

        for b in range(2):
            for i in range(N_TILES):
                tin = in_pool.tile([P, 3, F], DT, tag="in")
                nc.sync.dma_start(tin[:], pk[:, b, i])
                pt = tin[:, 0, :]
                tt = tin[:, 1, :]
                wt = tin[:, 2, :]

                # masks (DVE TS 4x):  m[:,0]=(t<0.1)  m[:,1]=(t>=0.3)
                m = m_pool.tile([P, 3, F], DT, tag="m")
                nc.vector.tensor_scalar(
                    m[:, 0, :], tt, THRESH_NEG, None, Alu.is_lt
                )
                nc.vector.tensor_scalar(
                    m[:, 1, :], tt, THRESH_POS, None, Alu.is_ge
                )
                # d = pred - target                          (DVE TT 2x)
                d = d_pool.tile([P, F], DT, tag="d")
                nc.vector.tensor_tensor(d[:], pt, tt, Alu.subtract)
                # l = d^2 into cols 0:96 of [24, 97] blocks  (ACT Square 1x)
                lext = lexts[(b * N_TILES + i) % 2]
                nc.scalar.activation(lext[:, :, 0:BD], d[:], Act.Square)
                # m[:,2] = m_pos * w                         (DVE TT 2x)
                nc.vector.tensor_tensor(m[:, 2, :], m[:, 1, :], wt, Alu.mult)

                # psum[b] += [l_blk | 1]^T @ [m_neg | m_pos | mw]   (PE)
                for k in range(NBLK):
                    sl = slice(k * BD, (k + 1) * BD)
                    nc.tensor.matmul(
                        psum[b][:, :, :],
                        lext[:, k, :],
                        m[:, :, sl],
                        start=(i == 0 and k == 0),
                        stop=(i == N_TILES - 1 and k == NBLK - 1),
                    )

            # dump the accumulated [97, 288] PSUM region to SBUF (ScalarE)
            nc.scalar.copy(ps_sb[:, b], psum[b][:, :, :])

        nc.sync.dma_start(out_ps[:], ps_sb[:])

    nc.compile()
    return nc


def _get_nc():
    global _compiled
    if _compiled is None:
        _compiled = _build_nc()
    return _compiled


def _np_branch_fallback(pred, target, weight):
    """Exact reference math in numpy float64 (handles k < num_neg)."""
    pred = pred.astype(np.float64)
    target = target.astype(np.float64)
    weight = weight.astype(np.float64)
    all_loss = (pred - target) ** 2
    pos_mask = (target >= THRESH_POS) & (weight != 0)
    neg_mask = target < THRESH_NEG
    pos_sum = float(np.sum(np.where(pos_mask, all_loss * weight, 0.0)))
    num_pos = int(np.sum(pos_mask))
    num_neg = int(np.sum(neg_mask))
    k = min(max(1000, 3 * num_pos), num_neg)
    neg_vals = all_loss[neg_mask]
    if k >= num_neg:
        topk = float(neg_vals.sum())
    elif k <= 0:
        topk = 0.0
    else:
        topk = float(np.partition(neg_vals, num_neg - k)[num_neg - k :].sum())
    return (pos_sum + topk) / (num_pos + k)


def kernel(output, character_map, affinity_map, character_weight, affinity_weight):
    from concourse.bass_utils import run_bass_kernel_spmd

    global LAST_RESULTS
    np_dt = ml_dtypes.bfloat16

    output = np.asarray(output, dtype=np.float32)

    def shard(a):
        # flat pixel order (b, h, w) -> [core, partition, tile, free]
        return (
            np.ascontiguousarray(a)
            .reshape(N_CORES, P, N_TILES, F)
            .astype(np_dt)
        )

    packed = np.empty((N_CORES, P, 2, N_TILES, 3, F), dtype=np_dt)
    packed[:, :, 0, :, 0] = shard(output[:, 0])
    packed[:, :, 0, :, 1] = shard(np.asarray(character_map, dtype=np.float32))
    packed[:, :, 0, :, 2] = shard(np.asarray(character_weight, dtype=np.float32))
    packed[:, :, 1, :, 0] = shard(output[:, 1])
    packed[:, :, 1, :, 1] = shard(np.asarray(affinity_map, dtype=np.float32))
    packed[:, :, 1, :, 2] = shard(np.asarray(affinity_weight, dtype=np.float32))

    in_maps = [{"pk": packed[c]} for c in range(N_CORES)]

    nc = _get_nc()
    res = run_bass_kernel_spmd(
        nc,
        in_maps,
        list(range(N_CORES)),
        trace=os.environ.get("KERNEL_TRACE", "0") == "1",
    )
    LAST_RESULTS = res

    # [cores, PR, branch, group, col]
    acc_ps = np.stack([r["acc_ps"] for r in res.results]).astype(np.float64)
    idx = np.arange(BD)
    s1 = acc_ps[:, idx, :, 0, idx].sum(axis=(0, 1))       # [branch]
    s2 = acc_ps[:, idx, :, 2, idx].sum(axis=(0, 1))       # [branch]
    n_neg = acc_ps[:, BD, :, 0, :].sum(axis=(0, 2))       # [branch]
    n_pos = acc_ps[:, BD, :, 1, :].sum(axis=(0, 2))       # [branch]

    total = 0.0
    for bidx, (tmap, wmap) in enumerate(
        [(character_map, character_weight), (affinity_map, affinity_weight)]
    ):
        num_neg = int(round(n_neg[bidx]))
        num_pos = int(round(n_pos[bidx]))
        k = min(max(1000, 3 * num_pos), num_neg)
        if k == num_neg:
            total += (s1[bidx] + s2[bidx]) / (num_pos + k)
        else:
            # top-k actually selective: fall back to exact host computation
            total += _np_branch_fallback(
                output[:, bidx].reshape(-1),
                np.asarray(tmap, dtype=np.float32).reshape(-1),
                np.asarray(wmap, dtype=np.float32).reshape(-1),
            )

    return np.float32(total)


# revision 14
# speedup vs baseline: 1.3517x; 1.0729x over previous
"""Trainium2 Bass kernel for the CRAFT-style hard-negative-mining MSE loss.

Reference math (per branch, over N = 16*768*768 flat pixels):
    all_loss = (pred - target)^2
    pos_mask = (target >= 0.3) & (weight != 0)
    neg_mask = (target < 0.1)
    pos_sum  = sum(pos_mask * all_loss * weight)
    k        = min(max(1000, 3*num_pos), num_neg)
    topk_sum = sum of k largest all_loss among negatives
    loss     = (pos_sum + topk_sum) / (num_pos + k)
    out      = loss_char + loss_aff

With uniform targets num_pos ~ 0.7*N, so 3*num_pos >> num_neg and
k == num_neg: the top-k degenerates to the full sum over negatives.

Device strategy (v3, TensorE-assisted): per 1/8 shard, per branch-tile
[128, F=2304]:
    DVE:  m_neg = (t < 0.1)            tensor_scalar is_lt  (4x mode)
          m_pos = (t >= 0.3)           tensor_scalar is_ge  (4x mode)
          d  = p - t                    tensor_tensor        (2x mode)
          mw = m_pos * w                tensor_tensor        (2x mode)
    ACT:  l  = d^2                      Square (1x) -> strided [24, 96]
    GPS:  ones column at lext[:, :, 96] (tiny memset)
    PE:   24 blocks of 96 data cols:
            psum[0:97, 0:288] += [l_96 | ones]^T @ [m_neg | m_pos | mw]
          accumulated over the whole branch (96 matmuls) into one
          [97, 288] PSUM region:
            diag(rows 0:96 of group 0) -> per-col <m_neg, l> = S1
            diag(rows 0:96 of group 2) -> per-col <mw,    l> = S2
            row 96 of group 0          -> per-col sums of m_neg = num_neg
            row 96 of group 1          -> per-col sums of m_pos = num_pos
The [97, 288] PSUM regions are copied to SBUF (ScalarE) and DMA'd out;
the host extracts diagonals/count-rows, sums across the 8 shards, and
applies the k/denominator logic (with a full numpy fallback for the
never-hit-here k < num_neg case).

This moves the masked-sum and count reductions off DVE/ACT (where they
only run at 1x) onto the otherwise-idle TensorE.  Inputs are cast to
bf16 on the host: halves HBM traffic and doubles DVE tensor_tensor
throughput.
"""

import os
import numpy as np
import ml_dtypes

N_CORES = 8
B, H, W = 16, 768, 768
NPX = B * H * W              # 9_437_184 flat pixels
P = 128                      # SBUF partitions
FD = NPX // (N_CORES * P)    # 9216 free-dim elements per core per tensor
N_TILES = 8                  # tiles per branch
F = FD // N_TILES            # 1152 tile width
BD = 96                      # data columns per matmul block
NBLK = F // BD               # 24 matmul blocks per tile
PR = BD + 1                  # psum rows used (96 data + 1 count row)

THRESH_NEG = 0.1
THRESH_POS = 0.3

_compiled = None             # cached nc
LAST_RESULTS = None          # BassKernelResults of the last run (for profiling)


def _build_nc():
    import concourse.bacc as bacc
    import concourse.mybir as mybir
    import concourse.tile as tile
    from contextlib import ExitStack

    DT = mybir.dt.bfloat16
    f32 = mybir.dt.float32
    Alu = mybir.AluOpType
    Act = mybir.ActivationFunctionType

    nc = bacc.Bacc(
        "TRN2",
        target_bir_lowering=False,
        debug=False,
        num_devices=N_CORES,
    )

    # packed input: [P, branch, tile, (p,t,w), F]
    pk = nc.declare_dram_parameter("pk", [P, 2, N_TILES, 3, F], DT, isOutput=False)
    out_ps = nc.declare_dram_parameter("acc_ps", [PR, 2, 3, BD], f32, isOutput=True)

    with tile.TileContext(nc) as tc, ExitStack() as ctx:
        in_pool = ctx.enter_context(tc.tile_pool(name="in", bufs=3))
        d_pool = ctx.enter_context(tc.tile_pool(name="d", bufs=2))
        m_pool = ctx.enter_context(tc.tile_pool(name="m", bufs=2))
        acc_pool = ctx.enter_context(tc.tile_pool(name="acc", bufs=1))
        ps_pool = ctx.enter_context(tc.psum_pool(name="ps", bufs=1))

        ps_sb = acc_pool.tile([PR, 2, 3, BD], f32, tag="ps_sb")
        psum = [
            ps_pool.tile([PR, 3, BD], f32, tag=f"psum{b}", name=f"psum{b}")
            for b in range(2)
        ]
        # persistent double-buffered [l | ones] stationary tensors; the
        # ones column (col 96 of each 97-block) is written once up front
        # and never touched again
        lexts = [
            acc_pool.tile([P, NBLK, PR], DT, tag=f"lext{j}", name=f"lext{j}")
            for j in range(2)
        ]
        for j in range(2):
            nc.gpsimd.memset(lexts[j][:, :, BD : BD + 1], 1.0)

        for b in range(2):
            for i in range(N_TILES):
                tin = in_pool.tile([P, 3, F], DT, tag="in")
                nc.sync.dma_start(tin[:], pk[:, b, i])
                pt = tin[:, 0, :]
                tt = tin[:, 1, :]
                wt = tin[:, 2, :]

                # d = pred - target first, so ACT can start  (DVE TT 2x)
                d = d_pool.tile([P, F], DT, tag="d")
                nc.vector.tensor_tensor(d[:], pt, tt, Alu.subtract)
                # l = d^2 into cols 0:96 of the 97-blocks, overlapping the
                # mask ops below                             (ACT Square 1x)
                lext = lexts[(b * N_TILES + i) % 2]
                nc.scalar.activation(lext[:, :, 0:BD], d[:], Act.Square)
                # masks (DVE TS 4x):  m[:,0]=(t<0.1)  m[:,1]=(t>=0.3)
                m = m_pool.tile([P, 3, F], DT, tag="m")
                nc.vector.tensor_scalar(
                    m[:, 0, :], tt, THRESH_NEG, None, Alu.is_lt
                )
                nc.vector.tensor_scalar(
                    m[:, 1, :], tt, THRESH_POS, None, Alu.is_ge
                )
                # m[:,2] = m_pos * w                         (DVE TT 2x)
                nc.vector.tensor_tensor(m[:, 2, :], m[:, 1, :], wt, Alu.mult)

                # psum[b] += [l_blk | 1]^T @ [m_neg | m_pos | mw]   (PE)
                for k in range(NBLK):
                    sl = slice(k * BD, (k + 1) * BD)
                    nc.tensor.matmul(
                        psum[b][:, :, :],
                        lext[:, k, :],
                        m[:, :, sl],
                        start=(i == 0 and k == 0),
                        stop=(i == N_TILES - 1 and k == NBLK - 1),
                    )

            # dump the accumulated [97, 288] PSUM region to SBUF (ScalarE),
            # then DMA it out in 4 partition slices on 4 different engine
            # queues (a single contiguous store serializes on one DMA ring)
            nc.scalar.copy(ps_sb[:, b], psum[b][:, :, :])
            slices = [(0, 25), (25, 49), (49, 73), (73, PR)]
            issuers = [nc.sync, nc.gpsimd, nc.scalar, nc.gpsimd]
            for (p0, p1), eng in zip(slices, issuers):
                eng.dma_start(out_ps[p0:p1, b], ps_sb[p0:p1, b])

    nc.compile()
    return nc


def _get_nc():
    global _compiled
    if _compiled is None:
        _compiled = _build_nc()
    return _compiled


def _np_branch_fallback(pred, target, weight):
    """Exact reference math in numpy float64 (handles k < num_neg)."""
    pred = pred.astype(np.float64)
    target = target.astype(np.float64)
    weight = weight.astype(np.float64)
    all_loss = (pred - target) ** 2
    pos_mask = (target >= THRESH_POS) & (weight != 0)
    neg_mask = target < THRESH_NEG
    pos_sum = float(np.sum(np.where(pos_mask, all_loss * weight, 0.0)))
    num_pos = int(np.sum(pos_mask))
    num_neg = int(np.sum(neg_mask))
    k = min(max(1000, 3 * num_pos), num_neg)
    neg_vals = all_loss[neg_mask]
    if k >= num_neg:
        topk = float(neg_vals.sum())
    elif k <= 0:
        topk = 0.0
    else:
        topk = float(np.partition(neg_vals, num_neg - k)[num_neg - k :].sum())
    return (pos_sum + topk) / (num_pos + k)


def kernel(output, character_map, affinity_map, character_weight, affinity_weight):
    from concourse.bass_utils import run_bass_kernel_spmd

    global LAST_RESULTS
    np_dt = ml_dtypes.bfloat16

    output = np.asarray(output, dtype=np.float32)

    def shard(a):
        # flat pixel order (b, h, w) -> [core, partition, tile, free]
        return (
            np.ascontiguousarray(a)
            .reshape(N_CORES, P, N_TILES, F)
            .astype(np_dt)
        )

    packed = np.empty((N_CORES, P, 2, N_TILES, 3, F), dtype=np_dt)
    packed[:, :, 0, :, 0] = shard(output[:, 0])
    packed[:, :, 0, :, 1] = shard(np.asarray(character_map, dtype=np.float32))
    packed[:, :, 0, :, 2] = shard(np.asarray(character_weight, dtype=np.float32))
    packed[:, :, 1, :, 0] = shard(output[:, 1])
    packed[:, :, 1, :, 1] = shard(np.asarray(affinity_map, dtype=np.float32))
    packed[:, :, 1, :, 2] = shard(np.asarray(affinity_weight, dtype=np.float32))

    in_maps = [{"pk": packed[c]} for c in range(N_CORES)]

    nc = _get_nc()
    res = run_bass_kernel_spmd(
        nc,
        in_maps,
        list(range(N_CORES)),
        trace=os.environ.get("KERNEL_TRACE", "0") == "1",
    )
    LAST_RESULTS = res

    # [cores, PR, branch, group, col]
    acc_ps = np.stack([r["acc_ps"] for r in res.results]).astype(np.float64)
    idx = np.arange(BD)
    s1 = acc_ps[:, idx, :, 0, idx].sum(axis=(0, 1))       # [branch]
    s2 = acc_ps[:, idx, :, 2, idx].sum(axis=(0, 1))       # [branch]
    n_neg = acc_ps[:, BD, :, 0, :].sum(axis=(0, 2))       # [branch]
    n_pos = acc_ps[:, BD, :, 1, :].sum(axis=(0, 2))       # [branch]

    total = 0.0
    for bidx, (tmap, wmap) in enumerate(
        [(character_map, character_weight), (affinity_map, affinity_weight)]
    ):
        num_neg = int(round(n_neg[bidx]))
        num_pos = int(round(n_pos[bidx]))
        k = min(max(1000, 3 * num_pos), num_neg)
        if k == num_neg:
            total += (s1[bidx] + s2[bidx]) / (num_pos + k)
        else:
            # top-k actually selective: fall back to exact host computation
            total += _np_branch_fallback(
                output[:, bidx].reshape(-1),
                np.asarray(tmap, dtype=np.float32).reshape(-1),
                np.asarray(wmap, dtype=np.float32).reshape(-1),
            )

    return np.float32(total)
